# revision 34
# baseline (speedup 1.0000x reference)
"""Trainium2 Bass kernel for nn_Attention_944892805701.

Dense transformer attention layer: QKV projection + RoPE + causal GQA SDPA +
output projection. B=2, S=2048, DIM=4096, 32 Q heads / 8 KV heads, hd=128.

Sharding (8 cores): 2 (batch) x 4 (head groups). Core (b, g) computes global
Q heads [8g, 8g+8) / KV heads [2g, 2g+2) of batch b and the corresponding
partial output projection y_partial = att_heads @ Wo[:, o_slice]^T. The host
sums the 4 head-group partials per batch (the tensor-parallel "allreduce",
done on host since full outputs are gathered there anyway).

Schedule (v2): per 512-wide chunk the PE stream is K-proj, Q0/Q1-proj,
V-proj (+PE transposes), then the flat per-(head, ktile) attention stream
with the remaining Q head-pair projections injected between early attention
items. Rope/acc drains (DVE) and exp (ACT) always have >=7us of queued PE
work behind them, so the PE never waits on them and stays at full clock.
Softmax denominators use the ones-matmul + deferred-normalize machinery.
Phase 3 (output projection) prefetches Wo tiles on the idle sync DMA ring
and drains PSUM via the Scalar engine; outT writes go out on the gpsimd
ring. bf16 matmul operands everywhere (fp8 fails the 2e-2 gate; measured
8e-2 end-to-end), fp32 PSUM accumulation, fp32 softmax statistics.
"""

import math
from contextlib import ExitStack

import numpy as np
import ml_dtypes

import concourse.bass as bass  # noqa: F401
import concourse.tile as tile
from concourse import bacc, mybir
from concourse.bass_utils import run_bass_kernel_spmd

F32 = mybir.dt.float32
F32R = mybir.dt.float32r
BF16 = mybir.dt.bfloat16

N_CORES = 8
DIM = 4096
N_HEADS = 32
N_KV_HEADS = 8
HEAD_DIM = 128
SEQ = 2048

HQ = N_HEADS // 4      # 8 local q heads
HKV = N_KV_HEADS // 4  # 2 local kv heads

SC = 512
P = 128
HH = HEAD_DIM // 2


def _r(ap):
    return ap.bitcast(F32R)


def build_program(seq=SEQ, dim=DIM, hq=HQ, hkv=HKV):
    nrep = hq // hkv
    nch = seq // SC
    ndt = dim // P
    nkt_total = seq // P
    dq = hq * HEAD_DIM
    dkv = hkv * HEAD_DIM
    scale = 1.0 / math.sqrt(HEAD_DIM)

    nc = bacc.Bacc("TRN2", target_bir_lowering=False, debug=False,
                   num_devices=N_CORES)

    xTd = nc.dram_tensor("xT", [dim, seq], BF16, kind="ExternalInput").ap()
    wqT = nc.dram_tensor("wqT", [dim, dq], BF16, kind="ExternalInput").ap()
    wkT = nc.dram_tensor("wkT", [dim, dkv], BF16, kind="ExternalInput").ap()
    wvT = nc.dram_tensor("wvT", [dim, dkv], BF16, kind="ExternalInput").ap()
    wot = nc.dram_tensor("wot", [dim // P, dq, P], BF16,
                         kind="ExternalInput").ap()
    cosT = nc.dram_tensor("cosT", [HEAD_DIM, seq], F32,
                          kind="ExternalInput").ap()
    # signed sin table: rows [0,64) hold -sin, rows [64,128) hold +sin, so
    # the rope drain is mul/mul/mul/add with no subtract.
    sinNT = nc.dram_tensor("sinNT", [HEAD_DIM, seq], F32,
                           kind="ExternalInput").ap()
    tri = nc.dram_tensor("tri", [P, P], BF16, kind="ExternalInput").ap()
    iden = nc.dram_tensor("iden", [P, P], BF16, kind="ExternalInput").ap()
    ones_col = nc.dram_tensor("ones_col", [P, 1], F32R,
                              kind="ExternalInput").ap()
    outT = nc.dram_tensor("outT", [dim, seq], F32, kind="ExternalOutput").ap()

    with ExitStack() as ctx:
        tc = ctx.enter_context(tile.TileContext(nc))
        ws = ctx.enter_context(tc.tile_pool(name="ws", bufs=14))    # f32 512
        csp = ctx.enter_context(tc.tile_pool(name="csp", bufs=4))   # cos/sin
        nrm = ctx.enter_context(tc.tile_pool(name="nrm", bufs=6))   # normalize
        wsb = ctx.enter_context(tc.tile_pool(name="wsb", bufs=95))  # bf16 512
        kp = ctx.enter_context(tc.tile_pool(name="kp", bufs=hkv))
        vp = ctx.enter_context(tc.tile_pool(name="vp", bufs=hkv * nkt_total))
        wqp = ctx.enter_context(tc.tile_pool(name="wqp", bufs=9))
        wkvp = ctx.enter_context(tc.tile_pool(name="wkvp", bufs=8))
        wop = ctx.enter_context(tc.tile_pool(name="wop", bufs=6))
        cns = ctx.enter_context(tc.tile_pool(name="cns", bufs=1))
        ps_a = ctx.enter_context(tc.tile_pool(name="ps_a", bufs=2,
                                              space="PSUM"))
        ps_p = ctx.enter_context(tc.tile_pool(name="ps_p", bufs=2,
                                              space="PSUM"))
        dram = ctx.enter_context(tc.tile_pool(name="dram", bufs=1,
                                              space="DRAM"))

        dn_dram = dram.tile([hq, seq], F32, tag="dn")
        dnr_dram = dram.tile([hq, seq], BF16, tag="dnr")

        tri_sb = cns.tile([P, P], BF16, tag="tri")
        nc.sync.dma_start(tri_sb[:], tri[:])
        iden_sb = cns.tile([P, P], BF16, tag="iden")
        nc.sync.dma_start(iden_sb[:], iden[:])
        ones_sb = cns.tile([P, 1], F32R, tag="ones")
        nc.sync.dma_start(ones_sb[:], ones_col[:])

        kTr = [kp.tile([P, seq], BF16, tag="k", name=f"kTr{g}")
               for g in range(hkv)]
        v_nat = [[vp.tile([P, HEAD_DIM], BF16, tag="v", name=f"v{g}_{t}")
                  for t in range(nkt_total)] for g in range(hkv)]
        # unnormalized attention output tiles, persistent through phase 3
        ao = [[wsb.tile([P, SC], BF16, tag="wsb", name=f"ao{h}_{cc}")
               for cc in range(nch)] for h in range(hq)]

        def rope_drain(dst, psum, cos_c, sinn_c):
            """dst(bf16) = psum*cos + rotate_half(psum)*sin, signed-sin."""
            tmp = ws.tile([P, SC], F32, tag="ws")
            nc.vector.tensor_mul(tmp[0:HH, :], psum[HH:P, :],
                                 sinn_c[0:HH, :])
            nc.vector.tensor_mul(tmp[HH:P, :], psum[0:HH, :],
                                 sinn_c[HH:P, :])
            nc.vector.tensor_mul(dst, psum, cos_c[:])
            nc.vector.tensor_add(dst, dst, tmp[:])

        def emit_loads(c):
            s0 = c * SC
            t = {}
            # chunk 0 is startup-DMA-bound: split weights onto the (idle)
            # gpsimd ring so xT and W stream in parallel.
            wring = nc.gpsimd if c == 0 else nc.sync
            # xT tiles interleaved with the wk quads (K projection is first
            # in the chunk's PE stream), then cos/sin (first needed at the
            # K rope drain), wq hb0 (Q01 is second), wv, then the remaining
            # wq pairs in consumption order.
            t["xT"] = [wsb.tile([P, SC], BF16, tag="wsb", name=f"xT{c}_{i}")
                       for i in range(ndt)]
            t["wk"] = []
            for dt4 in range(ndt // 4):
                for j in range(4):
                    dt = dt4 * 4 + j
                    nc.sync.dma_start(
                        t["xT"][dt][:],
                        xTd[dt * P:(dt + 1) * P, s0:s0 + SC])
                wk = wkvp.tile([P, 4, dkv], BF16, tag="wkv",
                               name=f"wk{c}_{dt4}")
                wring.dma_start(
                    wk[:], wkT[dt4 * 4 * P:(dt4 + 1) * 4 * P, :
                               ].rearrange("(d p) f -> p d f", p=P))
                t["wk"].append(wk)
            t["cos"] = csp.tile([P, SC], F32, tag="cs", name=f"cos{c}")
            nc.sync.dma_start(t["cos"][:], cosT[:, s0:s0 + SC])
            t["sin"] = csp.tile([P, SC], F32, tag="cs", name=f"sin{c}")
            nc.sync.dma_start(t["sin"][:], sinNT[:, s0:s0 + SC])
            t["wq"] = {}
            for hb in range(0, hq, 2):
                for dt4 in range(ndt // 4):
                    wq = wqp.tile([P, 4, 2 * HEAD_DIM], BF16, tag="wq",
                                  name=f"wq{c}_{hb}_{dt4}")
                    wring.dma_start(
                        wq[:], wqT[dt4 * 4 * P:(dt4 + 1) * 4 * P,
                                   hb * HEAD_DIM:(hb + 2) * HEAD_DIM
                                   ].rearrange("(d p) f -> p d f", p=P))
                    t["wq"][(hb, dt4)] = wq
                if hb == 0:
                    t["wv"] = []
                    for dt4 in range(ndt // 4):
                        wv = wkvp.tile([P, 4, dkv], BF16, tag="wkv",
                                       name=f"wv{c}_{dt4}")
                        wring.dma_start(
                            wv[:], wvT[dt4 * 4 * P:(dt4 + 1) * 4 * P, :
                                       ].rearrange("(d p) f -> p d f", p=P))
                        t["wv"].append(wv)
            return t

        def emit_q_pair(c, t, hb, qt):
            """Project q heads hb, hb+1 into per-chunk qt tiles."""
            for i in range(2):
                pq = ps_p.tile([P, SC], F32, tag="p",
                               name=f"pq{c}_{hb}_{i}")
                for dt in range(ndt):
                    wq = t["wq"][(hb, dt // 4)]
                    nc.tensor.matmul(
                        pq[:],
                        wq[:, dt % 4, i * HEAD_DIM:(i + 1) * HEAD_DIM],
                        t["xT"][dt][:],
                        start=(dt == 0), stop=(dt == ndt - 1))
                qt[hb + i] = wsb.tile([P, SC], BF16, tag="wsb",
                                      name=f"qt{c}_{hb + i}")
                rope_drain(qt[hb + i][:], pq[:], t["cos"], t["sin"])

        def emit_proj_k(c, t):
            s0 = c * SC
            for g in range(hkv):
                pk = ps_p.tile([P, SC], F32, tag="p", name=f"pk{c}_{g}")
                for dt in range(ndt):
                    wk = t["wk"][dt // 4]
                    nc.tensor.matmul(
                        pk[:],
                        wk[:, dt % 4, g * HEAD_DIM:(g + 1) * HEAD_DIM],
                        t["xT"][dt][:],
                        start=(dt == 0), stop=(dt == ndt - 1))
                rope_drain(kTr[g][:, s0:s0 + SC], pk[:], t["cos"], t["sin"])

        def emit_proj_v(c, t):
            for g in range(hkv):
                pv = ps_p.tile([P, SC], F32, tag="p", name=f"pv{c}_{g}")
                for dt in range(ndt):
                    wv = t["wv"][dt // 4]
                    nc.tensor.matmul(
                        pv[:],
                        wv[:, dt % 4, g * HEAD_DIM:(g + 1) * HEAD_DIM],
                        t["xT"][dt][:],
                        start=(dt == 0), stop=(dt == ndt - 1))
                vt_sb = wsb.tile([P, SC], BF16, tag="wsb")
                nc.scalar.copy(vt_sb[:], pv[:])
                for st in range(SC // P):
                    pt = ps_a.tile([P, P], BF16, tag="o", bufs=3)
                    nc.tensor.transpose(pt[:], vt_sb[:, st * P:(st + 1) * P],
                                        iden_sb[:])
                    nc.scalar.copy(v_nat[g][c * (SC // P) + st][:], pt[:])

        def emit_normalize_p1(c):
            """1/denom for chunk c + per-head rows to partition 0. DMAs on
            the scalar ring, reciprocal on DVE; no PE involvement."""
            s0 = c * SC
            dn_c = nrm.tile([P, SC], F32, tag="dnf", bufs=2, name=f"dnc{c}")
            nc.scalar.dma_start(dn_c[0:hq, :], dn_dram[:, s0:s0 + SC])
            rc_c = nrm.tile([P, SC], BF16, tag="dnb", bufs=2, name=f"rcc{c}")
            with nc.allow_low_precision(reason="softmax 1/denom in bf16"):
                nc.vector.reciprocal(rc_c[0:hq, :], dn_c[0:hq, :])
            nc.scalar.dma_start(dnr_dram[:, s0:s0 + SC], rc_c[0:hq, :])
            rrows = []
            for h in range(hq):
                rrow = nrm.tile([P, SC], BF16, tag="rr", bufs=6,
                                name=f"rrow{c}_{h}")
                nc.scalar.dma_start(rrow[0:1, :],
                                    dnr_dram[h:h + 1, s0:s0 + SC])
                rrows.append(rrow)
            return rrows

        def emit_normalize_p2(c, rrows):
            """Broadcast each 1/denom row via a PE outer product (ones_row
            [1,128] x rrow [1,512], 216ns) and scale ao in place on DVE."""
            for h in range(hq):
                rbp = ps_a.tile([P, SC], F32, tag="s", bufs=3,
                                name=f"rbp{c}_{h}")
                nc.tensor.matmul(rbp[:], tri_sb[0:1, :], rrows[h][0:1, :],
                                 start=True, stop=True)
                nc.vector.tensor_mul(ao[h][c][:], ao[h][c][:], rbp[:])

        def emit_attention(c, t, qt):
            s0 = c * SC
            nkt = (c + 1) * (SC // P)

            def emit_scores(h, g, kt):
                rr = kt * P - s0
                jlo = max(0, rr)
                pscr = ps_a.tile([P, SC], F32, tag="s", bufs=3,
                                 name=f"pscr{c}_{h}_{kt}")
                nc.tensor.matmul(
                    pscr[:, jlo:SC],
                    kTr[g][:, kt * P:(kt + 1) * P],
                    qt[h][:, jlo:SC],
                    start=True, stop=True)
                return pscr

            def emit_denom(h, acc):
                pd = ps_a.tile([P, SC], F32, tag="s", bufs=3,
                               name=f"pd{c}_{h}")
                nc.tensor.matmul(pd[0:1, :], ones_sb[:], _r(acc[:]),
                                 start=True, stop=True)
                dps = ws.tile([P, SC], F32, tag="ws", name=f"dps{c}_{h}")
                nc.scalar.copy(dps[0:1, :], pd[0:1, :])
                nc.scalar.dma_start(dn_dram[h:h + 1, s0:s0 + SC], dps[0:1, :])

            # remaining Q head-pair projections injected into the stream
            qproj_hooks = {(0, 1): 2, (2, 1): 4, (4, 1): 6}

            # flat (h, kt) stream with scores emitted 2 ahead across
            # head boundaries; denominators deferred into the next head
            items = [(h, kt) for h in range(hq) for kt in range(nkt)]
            pipe = {}

            def sc_ahead(i):
                h2, kt2 = items[i]
                pipe[i] = emit_scores(h2, h2 // nrep, kt2)

            sc_ahead(0)
            if len(items) > 1:
                sc_ahead(1)
            pending = None
            accs = {}
            pos = {}
            for i, (h, kt) in enumerate(items):
                g = h // nrep
                if kt == 0:
                    accs[h] = ws.tile([P, SC], F32, tag="ws",
                                      name=f"acc{c}_{h}")
                    pos[h] = ps_a.tile([P, SC], F32, tag="o", bufs=3,
                                       name=f"po{c}_{h}")
                acc, po = accs[h], pos[h]
                rr = kt * P - s0
                jlo = max(0, rr)
                if i + 2 < len(items):
                    sc_ahead(i + 2)
                hook = qproj_hooks.pop((h, kt), None)
                if hook is not None:
                    emit_q_pair(c, t, hook, qt)
                pscr = pipe.pop(i)
                if kt == 2 and pending is not None:
                    emit_denom(*pending)
                    pending = None
                et = wsb.tile([P, SC], BF16, tag="wsb",
                              name=f"et{c}_{h}_{kt}")
                nc.scalar.activation(
                    et[:, jlo:SC], pscr[:, jlo:SC],
                    mybir.ActivationFunctionType.Exp, scale=scale)
                if rr >= 0:
                    nc.vector.tensor_mul(et[:, jlo:jlo + P],
                                         et[:, jlo:jlo + P], tri_sb[:])
                if kt == 0:
                    nc.vector.tensor_copy(_r(acc[:]), et[:])
                else:
                    nc.vector.tensor_add(_r(acc[:, jlo:SC]),
                                         acc[:, jlo:SC], et[:, jlo:SC])
                nc.tensor.matmul(
                    po[:, jlo:SC],
                    v_nat[g][kt][:],
                    et[:, jlo:SC],
                    start=(kt == 0), stop=(kt == nkt - 1))
                if kt == nkt - 1:
                    if pending is not None:
                        emit_denom(*pending)
                    pending = (h, acc)
                    nc.scalar.copy(ao[h][c][:], po[:])
                    del accs[h], pos[h]
            if pending is not None:
                emit_denom(*pending)

        tiles = emit_loads(0)
        for c in range(nch):
            qt = [None] * hq
            emit_proj_k(c, tiles)
            rrows = emit_normalize_p1(c - 1) if c > 0 else None
            emit_q_pair(c, tiles, 0, qt)
            emit_proj_v(c, tiles)
            if rrows is not None:
                emit_normalize_p2(c - 1, rrows)
            nt = emit_loads(c + 1) if c + 1 < nch else None
            emit_attention(c, tiles, qt)
            tiles = nt

        # --- Phase 3: output projection from SBUF-resident att tiles.
        # Two passes (cc 0..2 then cc 3) so the final chunk's normalization
        # chain overlaps the first pass instead of stalling the PE. Wo tiles
        # prefetch on the (idle) sync DMA ring three iterations ahead.
        passes = ([0, 1], [2, 3]) if nch == 4 else (list(range(nch)),)
        wo_q = []

        def load_wo(m, p_i):
            wo = wop.tile([P, hq, P], BF16, tag="wo", name=f"wo{p_i}_{m}")
            nc.sync.dma_start(
                wo[:], wot[m].rearrange("(o p) f -> p o f", p=P))
            wo_q.append(wo)

        for p_i, cc_pass in enumerate(passes):
            if p_i == len(passes) - 1:
                # last chunk's normalize, hidden under the previous pass
                emit_normalize_p2(nch - 1, emit_normalize_p1(nch - 1))
            for m in range(3 if p_i == 0 else 0):
                load_wo(m, p_i)
            for m in range(dim // P):
                nm = m + 3
                if nm < dim // P:
                    load_wo(nm, p_i)
                elif p_i + 1 < len(passes):
                    load_wo(nm - dim // P, p_i + 1)
                wo = wo_q.pop(0)
                for cc in cc_pass:
                    # alternate free projection banks ("p") with score banks
                    # ("s") for a 6-bank pipeline; drains alternate engines.
                    par = (m + cc) % 2
                    if par == 0:
                        py = ps_p.tile([P, SC], F32, tag="p")
                    else:
                        py = ps_a.tile([P, SC], F32, tag="s", bufs=3)
                    for o in range(hq):
                        nc.tensor.matmul(
                            py[:], wo[:, o, :], ao[o][cc][:],
                            start=(o == 0), stop=(o == hq - 1))
                    yo = ws.tile([P, SC], F32, tag="ws")
                    if par == 0:
                        nc.vector.tensor_copy(yo[:], py[:])
                    else:
                        nc.scalar.copy(yo[:], py[:])
                    nc.gpsimd.dma_start(
                        outT[m * P:(m + 1) * P, cc * SC:(cc + 1) * SC], yo[:])

    nc.compile()
    return nc


def make_core_inputs(data, Wq, Wk, Wv, Wo, cos, sin):
    """Build in_maps for the 8 cores. Core id = 4*b + g."""
    bf = ml_dtypes.bfloat16

    def cbf(a):
        return np.ascontiguousarray(np.asarray(a).astype(bf))

    c = np.ascontiguousarray
    dq = HQ * HEAD_DIM
    dkv = HKV * HEAD_DIM
    dim = Wq.shape[1]
    tri_m = np.triu(np.ones((P, P), dtype=bf))
    iden = np.eye(P, dtype=bf)
    ones_col = np.ones((P, 1), dtype=np.float32)
    cosT = c(cos.T.astype(np.float32))
    sinNT = np.array(sin.T, dtype=np.float32)
    sinNT[0:HH, :] = -sinNT[0:HH, :]
    sinNT = c(sinNT)
    xt_by_batch = [cbf(data[b].T) for b in range(data.shape[0])]
    in_maps = []
    for core in range(N_CORES):
        b, g = divmod(core, 4)
        qs = slice(g * dq, (g + 1) * dq)
        ks = slice(g * dkv, (g + 1) * dkv)
        woT = Wo[:, qs].T                        # [dq, dim]
        wot = cbf(woT.reshape(dq, dim // P, P).transpose(1, 0, 2))
        in_maps.append({
            "xT": xt_by_batch[b],
            "wqT": cbf(Wq[qs, :].T),
            "wkT": cbf(Wk[ks, :].T),
            "wvT": cbf(Wv[ks, :].T),
            "wot": wot,
            "cosT": cosT,
            "sinNT": sinNT,
            "tri": tri_m,
            "iden": iden,
            "ones_col": ones_col,
        })
    return in_maps


_COMPILED = {}


def _get_program():
    key = (SEQ, DIM, HQ, HKV)
    if key not in _COMPILED:
        _COMPILED[key] = build_program()
    return _COMPILED[key]


def run(inputs, trace=False, tmpdir=None, trace_cores=None):
    nc = _get_program()
    in_maps = make_core_inputs(
        inputs["data"], inputs["Wq"], inputs["Wk"], inputs["Wv"],
        inputs["Wo"], inputs["cos"], inputs["sin"])
    kw = {}
    if trace:
        kw = dict(trace=True, tmpdir=tmpdir, trace_cores=trace_cores)
    res = run_bass_kernel_spmd(nc, in_maps, list(range(N_CORES)), **kw)
    B = inputs["data"].shape[0]
    out = np.zeros((B, SEQ, DIM), dtype=np.float32)
    for core in range(N_CORES):
        b = core // 4
        out[b] += res.results[core]["outT"].T
    return out, res


def kernel(data, Wq, Wk, Wv, Wo, cos, sin, mask):
    assert np.asarray(mask).size == 1, "only causal (numel==1) mask supported"
    inputs = {
        "data": np.asarray(data, dtype=np.float32),
        "Wq": np.asarray(Wq, dtype=np.float32),
        "Wk": np.asarray(Wk, dtype=np.float32),
        "Wv": np.asarray(Wv, dtype=np.float32),
        "Wo": np.asarray(Wo, dtype=np.float32),
        "cos": np.asarray(cos, dtype=np.float32),
        "sin": np.asarray(sin, dtype=np.float32),
    }
    out, _ = run(inputs)
    return out


# revision 35
# speedup vs baseline: 1.0190x; 1.0190x over previous
"""Trainium2 Bass kernel for nn_Attention_944892805701.

Dense transformer attention layer: QKV projection + RoPE + causal GQA SDPA +
output projection. B=2, S=2048, DIM=4096, 32 Q heads / 8 KV heads, hd=128.

Sharding (8 cores): 2 (batch) x 4 (head groups). Core (b, g) computes global
Q heads [8g, 8g+8) / KV heads [2g, 2g+2) of batch b and the corresponding
partial output projection y_partial = att_heads @ Wo[:, o_slice]^T. The host
sums the 4 head-group partials per batch (the tensor-parallel "allreduce",
done on host since full outputs are gathered there anyway).

Schedule (v2): per 512-wide chunk the PE stream is K-proj, Q0/Q1-proj,
V-proj (+PE transposes), then the flat per-(head, ktile) attention stream
with the remaining Q head-pair projections injected between early attention
items. Rope/acc drains (DVE) and exp (ACT) always have >=7us of queued PE
work behind them, so the PE never waits on them and stays at full clock.
Softmax denominators use the ones-matmul + deferred-normalize machinery.
Phase 3 (output projection) prefetches Wo tiles on the idle sync DMA ring
and drains PSUM via the Scalar engine; outT writes go out on the gpsimd
ring. bf16 matmul operands everywhere (fp8 fails the 2e-2 gate; measured
8e-2 end-to-end), fp32 PSUM accumulation, fp32 softmax statistics.
"""

import math
from contextlib import ExitStack

import numpy as np
import ml_dtypes

import concourse.bass as bass  # noqa: F401
import concourse.tile as tile
from concourse import bacc, mybir
from concourse.bass_utils import run_bass_kernel_spmd

F32 = mybir.dt.float32
F32R = mybir.dt.float32r
BF16 = mybir.dt.bfloat16

N_CORES = 8
DIM = 4096
N_HEADS = 32
N_KV_HEADS = 8
HEAD_DIM = 128
SEQ = 2048

HQ = N_HEADS // 4      # 8 local q heads
HKV = N_KV_HEADS // 4  # 2 local kv heads

SC = 512
P = 128
HH = HEAD_DIM // 2


def _r(ap):
    return ap.bitcast(F32R)


def build_program(seq=SEQ, dim=DIM, hq=HQ, hkv=HKV):
    nrep = hq // hkv
    nch = seq // SC
    ndt = dim // P
    nkt_total = seq // P
    dq = hq * HEAD_DIM
    dkv = hkv * HEAD_DIM
    scale = 1.0 / math.sqrt(HEAD_DIM)

    nc = bacc.Bacc("TRN2", target_bir_lowering=False, debug=False,
                   num_devices=N_CORES)

    xTd = nc.dram_tensor("xT", [dim, seq], BF16, kind="ExternalInput").ap()
    wqT = nc.dram_tensor("wqT", [dim, dq], BF16, kind="ExternalInput").ap()
    wkT = nc.dram_tensor("wkT", [dim, dkv], BF16, kind="ExternalInput").ap()
    wvT = nc.dram_tensor("wvT", [dim, dkv], BF16, kind="ExternalInput").ap()
    wot = nc.dram_tensor("wot", [dim // P, dq, P], BF16,
                         kind="ExternalInput").ap()
    cosT = nc.dram_tensor("cosT", [HEAD_DIM, seq], F32,
                          kind="ExternalInput").ap()
    # signed sin table: rows [0,64) hold -sin, rows [64,128) hold +sin, so
    # the rope drain is mul/mul/mul/add with no subtract.
    sinNT = nc.dram_tensor("sinNT", [HEAD_DIM, seq], F32,
                           kind="ExternalInput").ap()
    tri = nc.dram_tensor("tri", [P, P], BF16, kind="ExternalInput").ap()
    iden = nc.dram_tensor("iden", [P, P], BF16, kind="ExternalInput").ap()
    ones_col = nc.dram_tensor("ones_col", [P, 1], F32R,
                              kind="ExternalInput").ap()
    outT = nc.dram_tensor("outT", [dim, seq], F32, kind="ExternalOutput").ap()

    with ExitStack() as ctx:
        tc = ctx.enter_context(tile.TileContext(nc))
        ws = ctx.enter_context(tc.tile_pool(name="ws", bufs=14))    # f32 512
        csp = ctx.enter_context(tc.tile_pool(name="csp", bufs=4))   # cos/sin
        nrm = ctx.enter_context(tc.tile_pool(name="nrm", bufs=6))   # normalize
        wsb = ctx.enter_context(tc.tile_pool(name="wsb", bufs=95))  # bf16 512
        kp = ctx.enter_context(tc.tile_pool(name="kp", bufs=hkv))
        vp = ctx.enter_context(tc.tile_pool(name="vp", bufs=hkv * nkt_total))
        wqp = ctx.enter_context(tc.tile_pool(name="wqp", bufs=9))
        wkvp = ctx.enter_context(tc.tile_pool(name="wkvp", bufs=8))
        wop = ctx.enter_context(tc.tile_pool(name="wop", bufs=6))
        cns = ctx.enter_context(tc.tile_pool(name="cns", bufs=1))
        ps_a = ctx.enter_context(tc.tile_pool(name="ps_a", bufs=2,
                                              space="PSUM"))
        ps_p = ctx.enter_context(tc.tile_pool(name="ps_p", bufs=2,
                                              space="PSUM"))
        dram = ctx.enter_context(tc.tile_pool(name="dram", bufs=1,
                                              space="DRAM"))

        dn_dram = dram.tile([hq, seq], F32, tag="dn")
        dnr_dram = dram.tile([hq, seq], BF16, tag="dnr")

        tri_sb = cns.tile([P, P], BF16, tag="tri")
        nc.sync.dma_start(tri_sb[:], tri[:])
        iden_sb = cns.tile([P, P], BF16, tag="iden")
        nc.sync.dma_start(iden_sb[:], iden[:])
        ones_sb = cns.tile([P, 1], F32R, tag="ones")
        nc.sync.dma_start(ones_sb[:], ones_col[:])

        kTr = [kp.tile([P, seq], BF16, tag="k", name=f"kTr{g}")
               for g in range(hkv)]
        v_nat = [[vp.tile([P, HEAD_DIM], BF16, tag="v", name=f"v{g}_{t}")
                  for t in range(nkt_total)] for g in range(hkv)]
        # unnormalized attention output tiles, persistent through phase 3
        ao = [[wsb.tile([P, SC], BF16, tag="wsb", name=f"ao{h}_{cc}")
               for cc in range(nch)] for h in range(hq)]

        def rope_drain(dst, psum, cos_c, sinn_c):
            """dst(bf16) = psum*cos + rotate_half(psum)*sin, signed-sin."""
            tmp = ws.tile([P, SC], F32, tag="ws")
            nc.vector.tensor_mul(tmp[0:HH, :], psum[HH:P, :],
                                 sinn_c[0:HH, :])
            nc.vector.tensor_mul(tmp[HH:P, :], psum[0:HH, :],
                                 sinn_c[HH:P, :])
            nc.vector.tensor_mul(dst, psum, cos_c[:])
            nc.vector.tensor_add(dst, dst, tmp[:])

        def emit_loads(c):
            s0 = c * SC
            t = {}
            # chunk 0 is startup-DMA-bound: split weights onto the (idle)
            # gpsimd ring so xT and W stream in parallel.
            wring = nc.gpsimd if c == 0 else nc.sync
            # xT tiles interleaved with the wk quads (K projection is first
            # in the chunk's PE stream), then cos/sin (first needed at the
            # K rope drain), wq hb0 (Q01 is second), wv, then the remaining
            # wq pairs in consumption order.
            t["xT"] = [wsb.tile([P, SC], BF16, tag="wsb", name=f"xT{c}_{i}")
                       for i in range(ndt)]
            t["wk"] = []
            for dt4 in range(ndt // 4):
                for j in range(4):
                    dt = dt4 * 4 + j
                    nc.sync.dma_start(
                        t["xT"][dt][:],
                        xTd[dt * P:(dt + 1) * P, s0:s0 + SC])
                wk = wkvp.tile([P, 4, dkv], BF16, tag="wkv",
                               name=f"wk{c}_{dt4}")
                wring.dma_start(
                    wk[:], wkT[dt4 * 4 * P:(dt4 + 1) * 4 * P, :
                               ].rearrange("(d p) f -> p d f", p=P))
                t["wk"].append(wk)
            t["cos"] = csp.tile([P, SC], F32, tag="cs", name=f"cos{c}")
            nc.sync.dma_start(t["cos"][:], cosT[:, s0:s0 + SC])
            t["sin"] = csp.tile([P, SC], F32, tag="cs", name=f"sin{c}")
            nc.sync.dma_start(t["sin"][:], sinNT[:, s0:s0 + SC])
            t["wq"] = {}
            for hb in range(0, hq, 2):
                for dt4 in range(ndt // 4):
                    wq = wqp.tile([P, 4, 2 * HEAD_DIM], BF16, tag="wq",
                                  name=f"wq{c}_{hb}_{dt4}")
                    wring.dma_start(
                        wq[:], wqT[dt4 * 4 * P:(dt4 + 1) * 4 * P,
                                   hb * HEAD_DIM:(hb + 2) * HEAD_DIM
                                   ].rearrange("(d p) f -> p d f", p=P))
                    t["wq"][(hb, dt4)] = wq
                if hb == 0:
                    t["wv"] = []
                    for dt4 in range(ndt // 4):
                        wv = wkvp.tile([P, 4, dkv], BF16, tag="wkv",
                                       name=f"wv{c}_{dt4}")
                        wring.dma_start(
                            wv[:], wvT[dt4 * 4 * P:(dt4 + 1) * 4 * P, :
                                       ].rearrange("(d p) f -> p d f", p=P))
                        t["wv"].append(wv)
            return t

        def emit_q_pair(c, t, hb, qt):
            """Project q heads hb, hb+1 into per-chunk qt tiles."""
            for i in range(2):
                pq = ps_p.tile([P, SC], F32, tag="p",
                               name=f"pq{c}_{hb}_{i}")
                for dt in range(ndt):
                    wq = t["wq"][(hb, dt // 4)]
                    nc.tensor.matmul(
                        pq[:],
                        wq[:, dt % 4, i * HEAD_DIM:(i + 1) * HEAD_DIM],
                        t["xT"][dt][:],
                        start=(dt == 0), stop=(dt == ndt - 1))
                qt[hb + i] = wsb.tile([P, SC], BF16, tag="wsb",
                                      name=f"qt{c}_{hb + i}")
                rope_drain(qt[hb + i][:], pq[:], t["cos"], t["sin"])

        def emit_proj_k(c, t):
            s0 = c * SC
            for g in range(hkv):
                pk = ps_p.tile([P, SC], F32, tag="p", name=f"pk{c}_{g}")
                for dt in range(ndt):
                    wk = t["wk"][dt // 4]
                    nc.tensor.matmul(
                        pk[:],
                        wk[:, dt % 4, g * HEAD_DIM:(g + 1) * HEAD_DIM],
                        t["xT"][dt][:],
                        start=(dt == 0), stop=(dt == ndt - 1))
                rope_drain(kTr[g][:, s0:s0 + SC], pk[:], t["cos"], t["sin"])

        def emit_proj_v(c, t):
            for g in range(hkv):
                pv = ps_p.tile([P, SC], F32, tag="p", name=f"pv{c}_{g}")
                for dt in range(ndt):
                    wv = t["wv"][dt // 4]
                    nc.tensor.matmul(
                        pv[:],
                        wv[:, dt % 4, g * HEAD_DIM:(g + 1) * HEAD_DIM],
                        t["xT"][dt][:],
                        start=(dt == 0), stop=(dt == ndt - 1))
                vt_sb = wsb.tile([P, SC], BF16, tag="wsb")
                nc.scalar.copy(vt_sb[:], pv[:])
                for st in range(SC // P):
                    pt = ps_a.tile([P, P], BF16, tag="o", bufs=3)
                    nc.tensor.transpose(pt[:], vt_sb[:, st * P:(st + 1) * P],
                                        iden_sb[:])
                    nc.scalar.copy(v_nat[g][c * (SC // P) + st][:], pt[:])

        def emit_normalize_p1(c):
            """1/denom for chunk c + per-head rows to partition 0. DMAs on
            the scalar ring, reciprocal on DVE; no PE involvement."""
            s0 = c * SC
            dn_c = nrm.tile([P, SC], F32, tag="dnf", bufs=2, name=f"dnc{c}")
            nc.scalar.dma_start(dn_c[0:hq, :], dn_dram[:, s0:s0 + SC])
            rc_c = nrm.tile([P, SC], BF16, tag="dnb", bufs=2, name=f"rcc{c}")
            with nc.allow_low_precision(reason="softmax 1/denom in bf16"):
                nc.vector.reciprocal(rc_c[0:hq, :], dn_c[0:hq, :])
            nc.scalar.dma_start(dnr_dram[:, s0:s0 + SC], rc_c[0:hq, :])
            rrows = []
            for h in range(hq):
                rrow = nrm.tile([P, SC], BF16, tag="rr", bufs=6,
                                name=f"rrow{c}_{h}")
                nc.scalar.dma_start(rrow[0:1, :],
                                    dnr_dram[h:h + 1, s0:s0 + SC])
                rrows.append(rrow)
            return rrows

        def emit_normalize_p2(c, rrows):
            """Broadcast each 1/denom row via a PE outer product (ones_row
            [1,128] x rrow [1,512], 216ns) and scale ao in place on DVE."""
            for h in range(hq):
                rbp = ps_a.tile([P, SC], F32, tag="s", bufs=3,
                                name=f"rbp{c}_{h}")
                nc.tensor.matmul(rbp[:], tri_sb[0:1, :], rrows[h][0:1, :],
                                 start=True, stop=True)
                nc.vector.tensor_mul(ao[h][c][:], ao[h][c][:], rbp[:])

        def emit_attention(c, t, qt):
            s0 = c * SC
            nkt = (c + 1) * (SC // P)

            def emit_scores(h, g, kt):
                rr = kt * P - s0
                jlo = max(0, rr)
                pscr = ps_a.tile([P, SC], F32, tag="s", bufs=3,
                                 name=f"pscr{c}_{h}_{kt}")
                nc.tensor.matmul(
                    pscr[:, jlo:SC],
                    kTr[g][:, kt * P:(kt + 1) * P],
                    qt[h][:, jlo:SC],
                    start=True, stop=True)
                return pscr

            def emit_denom(h, acc):
                pd = ps_a.tile([P, SC], F32, tag="s", bufs=3,
                               name=f"pd{c}_{h}")
                nc.tensor.matmul(pd[0:1, :], ones_sb[:], _r(acc[:]),
                                 start=True, stop=True)
                dps = ws.tile([P, SC], F32, tag="ws", name=f"dps{c}_{h}")
                nc.scalar.copy(dps[0:1, :], pd[0:1, :])
                nc.scalar.dma_start(dn_dram[h:h + 1, s0:s0 + SC], dps[0:1, :])

            # remaining Q head-pair projections injected into the stream
            qproj_hooks = {(0, 1): 2, (2, 1): 4, (4, 1): 6}

            # flat (h, kt) stream with scores emitted 2 ahead across
            # head boundaries; denominators deferred into the next head
            items = [(h, kt) for h in range(hq) for kt in range(nkt)]
            pipe = {}

            def sc_ahead(i):
                h2, kt2 = items[i]
                pipe[i] = emit_scores(h2, h2 // nrep, kt2)

            sc_ahead(0)
            if len(items) > 1:
                sc_ahead(1)
            pending = None
            accs = {}
            pos = {}
            for i, (h, kt) in enumerate(items):
                g = h // nrep
                if kt == 0:
                    accs[h] = ws.tile([P, SC], F32, tag="ws",
                                      name=f"acc{c}_{h}")
                    pos[h] = ps_a.tile([P, SC], F32, tag="o", bufs=3,
                                       name=f"po{c}_{h}")
                acc, po = accs[h], pos[h]
                rr = kt * P - s0
                jlo = max(0, rr)
                if i + 2 < len(items):
                    sc_ahead(i + 2)
                hook = qproj_hooks.pop((h, kt), None)
                if hook is not None:
                    emit_q_pair(c, t, hook, qt)
                pscr = pipe.pop(i)
                if kt == 2 and pending is not None:
                    emit_denom(*pending)
                    pending = None
                et = wsb.tile([P, SC], BF16, tag="wsb",
                              name=f"et{c}_{h}_{kt}")
                nc.scalar.activation(
                    et[:, jlo:SC], pscr[:, jlo:SC],
                    mybir.ActivationFunctionType.Exp, scale=scale)
                if rr >= 0:
                    nc.vector.tensor_mul(et[:, jlo:jlo + P],
                                         et[:, jlo:jlo + P], tri_sb[:])
                if kt == 0:
                    nc.vector.tensor_copy(_r(acc[:]), et[:])
                else:
                    nc.vector.tensor_add(_r(acc[:, jlo:SC]),
                                         acc[:, jlo:SC], et[:, jlo:SC])
                nc.tensor.matmul(
                    po[:, jlo:SC],
                    v_nat[g][kt][:],
                    et[:, jlo:SC],
                    start=(kt == 0), stop=(kt == nkt - 1))
                if kt == nkt - 1:
                    if pending is not None:
                        emit_denom(*pending)
                    pending = (h, acc)
                    nc.scalar.copy(ao[h][c][:], po[:])
                    del accs[h], pos[h]
            if pending is not None:
                emit_denom(*pending)

        tiles = emit_loads(0)
        for c in range(nch):
            qt = [None] * hq
            emit_proj_k(c, tiles)
            rrows = emit_normalize_p1(c - 1) if c > 0 else None
            emit_q_pair(c, tiles, 0, qt)
            emit_proj_v(c, tiles)
            if rrows is not None:
                emit_normalize_p2(c - 1, rrows)
            nt = emit_loads(c + 1) if c + 1 < nch else None
            emit_attention(c, tiles, qt)
            tiles = nt

        # --- Phase 3: output projection from SBUF-resident att tiles.
        # Two passes (cc 0..2 then cc 3) so the final chunk's normalization
        # chain overlaps the first pass instead of stalling the PE. Wo tiles
        # prefetch on the (idle) sync DMA ring three iterations ahead.
        passes = ([0, 1], [2, 3]) if nch == 4 else (list(range(nch)),)
        wo_q = []

        def load_wo(m, p_i):
            wo = wop.tile([P, hq, P], BF16, tag="wo", name=f"wo{p_i}_{m}")
            nc.sync.dma_start(
                wo[:], wot[m].rearrange("(o p) f -> p o f", p=P))
            wo_q.append(wo)

        for p_i, cc_pass in enumerate(passes):
            if p_i == len(passes) - 1:
                # last chunk's normalize, hidden under the previous pass
                emit_normalize_p2(nch - 1, emit_normalize_p1(nch - 1))
            for m in range(3 if p_i == 0 else 0):
                load_wo(m, p_i)
            for m in range(dim // P):
                nm = m + 3
                if nm < dim // P:
                    load_wo(nm, p_i)
                elif p_i + 1 < len(passes):
                    load_wo(nm - dim // P, p_i + 1)
                wo = wo_q.pop(0)
                for cc in cc_pass:
                    # alternate free projection banks ("p") with score banks
                    # ("s") for a 6-bank pipeline; drains alternate engines.
                    par = (m + cc) % 2
                    if par == 0:
                        py = ps_p.tile([P, SC], F32, tag="p")
                    else:
                        py = ps_a.tile([P, SC], F32, tag="s", bufs=3)
                    for o in range(hq):
                        nc.tensor.matmul(
                            py[:], wo[:, o, :], ao[o][cc][:],
                            start=(o == 0), stop=(o == hq - 1))
                    yo = ws.tile([P, SC], F32, tag="ws")
                    if par == 0:
                        nc.vector.tensor_copy(yo[:], py[:])
                    else:
                        nc.scalar.copy(yo[:], py[:])
                    nc.sync.dma_start(
                        outT[m * P:(m + 1) * P, cc * SC:(cc + 1) * SC], yo[:])

    nc.compile()
    return nc


def make_core_inputs(data, Wq, Wk, Wv, Wo, cos, sin):
    """Build in_maps for the 8 cores. Core id = 4*b + g."""
    bf = ml_dtypes.bfloat16

    def cbf(a):
        return np.ascontiguousarray(np.asarray(a).astype(bf))

    c = np.ascontiguousarray
    dq = HQ * HEAD_DIM
    dkv = HKV * HEAD_DIM
    dim = Wq.shape[1]
    tri_m = np.triu(np.ones((P, P), dtype=bf))
    iden = np.eye(P, dtype=bf)
    ones_col = np.ones((P, 1), dtype=np.float32)
    cosT = c(cos.T.astype(np.float32))
    sinNT = np.array(sin.T, dtype=np.float32)
    sinNT[0:HH, :] = -sinNT[0:HH, :]
    sinNT = c(sinNT)
    xt_by_batch = [cbf(data[b].T) for b in range(data.shape[0])]
    in_maps = []
    for core in range(N_CORES):
        b, g = divmod(core, 4)
        qs = slice(g * dq, (g + 1) * dq)
        ks = slice(g * dkv, (g + 1) * dkv)
        woT = Wo[:, qs].T                        # [dq, dim]
        wot = cbf(woT.reshape(dq, dim // P, P).transpose(1, 0, 2))
        in_maps.append({
            "xT": xt_by_batch[b],
            "wqT": cbf(Wq[qs, :].T),
            "wkT": cbf(Wk[ks, :].T),
            "wvT": cbf(Wv[ks, :].T),
            "wot": wot,
            "cosT": cosT,
            "sinNT": sinNT,
            "tri": tri_m,
            "iden": iden,
            "ones_col": ones_col,
        })
    return in_maps


_COMPILED = {}


def _get_program():
    key = (SEQ, DIM, HQ, HKV)
    if key not in _COMPILED:
        _COMPILED[key] = build_program()
    return _COMPILED[key]


def run(inputs, trace=False, tmpdir=None, trace_cores=None):
    nc = _get_program()
    in_maps = make_core_inputs(
        inputs["data"], inputs["Wq"], inputs["Wk"], inputs["Wv"],
        inputs["Wo"], inputs["cos"], inputs["sin"])
    kw = {}
    if trace:
        kw = dict(trace=True, tmpdir=tmpdir, trace_cores=trace_cores)
    res = run_bass_kernel_spmd(nc, in_maps, list(range(N_CORES)), **kw)
    B = inputs["data"].shape[0]
    out = np.zeros((B, SEQ, DIM), dtype=np.float32)
    for core in range(N_CORES):
        b = core // 4
        out[b] += res.results[core]["outT"].T
    return out, res


def kernel(data, Wq, Wk, Wv, Wo, cos, sin, mask):
    assert np.asarray(mask).size == 1, "only causal (numel==1) mask supported"
    inputs = {
        "data": np.asarray(data, dtype=np.float32),
        "Wq": np.asarray(Wq, dtype=np.float32),
        "Wk": np.asarray(Wk, dtype=np.float32),
        "Wv": np.asarray(Wv, dtype=np.float32),
        "Wo": np.asarray(Wo, dtype=np.float32),
        "cos": np.asarray(cos, dtype=np.float32),
        "sin": np.asarray(sin, dtype=np.float32),
    }
    out, _ = run(inputs)
    return out


# revision 37
# speedup vs baseline: 1.0465x; 1.0270x over previous
"""Trainium2 Bass kernel for nn_Attention_944892805701.

Dense transformer attention layer: QKV projection + RoPE + causal GQA SDPA +
output projection. B=2, S=2048, DIM=4096, 32 Q heads / 8 KV heads, hd=128.

Sharding (8 cores): 2 (batch) x 4 (head groups). Core (b, g) computes global
Q heads [8g, 8g+8) / KV heads [2g, 2g+2) of batch b and the corresponding
partial output projection y_partial = att_heads @ Wo[:, o_slice]^T. The host
sums the 4 head-group partials per batch (the tensor-parallel "allreduce",
done on host since full outputs are gathered there anyway).

Schedule: per 512-wide chunk the PE stream is K-proj, Q0/Q1-proj, V-proj
(+PE transposes), then the flat per-(head, ktile) attention stream with the
remaining Q head-pair projections injected between early attention items,
so rope/acc drains (DVE) and exp (ACT) always have queued PE work behind
them and the PE stays at full clock. Softmax denominators: f32r ones-matmul
per head, deferred; the per-chunk normalize runs one chunk later — dn/dnr
roundtrip + reciprocal off the critical path (scalar DMA ring + DVE), and
the 1/denom broadcast is a PE outer product (ones-row x row, 216ns) into
PSUM followed by an in-place DVE multiply — no gpsimd in any latency chain
(its ISA ops + DMA ring measured 15us+/op in dependent chains). Phase 3
(output projection) runs two cc-passes with Wo tiles prefetched 3 ahead on
the sync ring; PSUM drains alternate Scalar/Vector. bf16 matmul operands
everywhere (fp8 fails the 2e-2 gate: measured 8.2e-2 end-to-end for fp8
QKV inputs alone), fp32 PSUM accumulation, fp32 softmax statistics.

Measured (8 cores, full clock): ~808-835 us vs 1019-1031 us baseline,
rel err 7.7e-3. Note: the device DVFS state varies run-to-run (some runs
execute all matmuls ~20% slower); compare kernels only across several runs.
"""

import math
from contextlib import ExitStack

import numpy as np
import ml_dtypes

import concourse.bass as bass  # noqa: F401
import concourse.tile as tile
from concourse import bacc, mybir
from concourse.bass_utils import run_bass_kernel_spmd

F32 = mybir.dt.float32
F32R = mybir.dt.float32r
BF16 = mybir.dt.bfloat16

N_CORES = 8
DIM = 4096
N_HEADS = 32
N_KV_HEADS = 8
HEAD_DIM = 128
SEQ = 2048

HQ = N_HEADS // 4      # 8 local q heads
HKV = N_KV_HEADS // 4  # 2 local kv heads

SC = 512
P = 128
HH = HEAD_DIM // 2


def _r(ap):
    return ap.bitcast(F32R)


def build_program(seq=SEQ, dim=DIM, hq=HQ, hkv=HKV):
    nrep = hq // hkv
    nch = seq // SC
    ndt = dim // P
    nkt_total = seq // P
    dq = hq * HEAD_DIM
    dkv = hkv * HEAD_DIM
    scale = 1.0 / math.sqrt(HEAD_DIM)

    nc = bacc.Bacc("TRN2", target_bir_lowering=False, debug=False,
                   num_devices=N_CORES)

    xTd = nc.dram_tensor("xT", [dim, seq], BF16, kind="ExternalInput").ap()
    wqT = nc.dram_tensor("wqT", [dim, dq], BF16, kind="ExternalInput").ap()
    wkT = nc.dram_tensor("wkT", [dim, dkv], BF16, kind="ExternalInput").ap()
    wvT = nc.dram_tensor("wvT", [dim, dkv], BF16, kind="ExternalInput").ap()
    wot = nc.dram_tensor("wot", [dim // P, dq, P], BF16,
                         kind="ExternalInput").ap()
    cosT = nc.dram_tensor("cosT", [HEAD_DIM, seq], F32,
                          kind="ExternalInput").ap()
    # signed sin table: rows [0,64) hold -sin, rows [64,128) hold +sin, so
    # the rope drain is mul/mul/mul/add with no subtract.
    sinNT = nc.dram_tensor("sinNT", [HEAD_DIM, seq], F32,
                           kind="ExternalInput").ap()
    tri = nc.dram_tensor("tri", [P, P], BF16, kind="ExternalInput").ap()
    iden = nc.dram_tensor("iden", [P, P], BF16, kind="ExternalInput").ap()
    ones_col = nc.dram_tensor("ones_col", [P, 1], F32R,
                              kind="ExternalInput").ap()
    outT = nc.dram_tensor("outT", [dim, seq], F32, kind="ExternalOutput").ap()

    with ExitStack() as ctx:
        tc = ctx.enter_context(tile.TileContext(nc))
        ws = ctx.enter_context(tc.tile_pool(name="ws", bufs=14))    # f32 512
        csp = ctx.enter_context(tc.tile_pool(name="csp", bufs=4))   # cos/sin
        nrm = ctx.enter_context(tc.tile_pool(name="nrm", bufs=6))   # normalize
        wsb = ctx.enter_context(tc.tile_pool(name="wsb", bufs=95))  # bf16 512
        kp = ctx.enter_context(tc.tile_pool(name="kp", bufs=hkv))
        vp = ctx.enter_context(tc.tile_pool(name="vp", bufs=hkv * nkt_total))
        wqp = ctx.enter_context(tc.tile_pool(name="wqp", bufs=9))
        wkvp = ctx.enter_context(tc.tile_pool(name="wkvp", bufs=8))
        wop = ctx.enter_context(tc.tile_pool(name="wop", bufs=6))
        cns = ctx.enter_context(tc.tile_pool(name="cns", bufs=1))
        ps_a = ctx.enter_context(tc.tile_pool(name="ps_a", bufs=2,
                                              space="PSUM"))
        ps_p = ctx.enter_context(tc.tile_pool(name="ps_p", bufs=2,
                                              space="PSUM"))
        dram = ctx.enter_context(tc.tile_pool(name="dram", bufs=1,
                                              space="DRAM"))

        dn_dram = dram.tile([hq, seq], F32, tag="dn")
        dnr_dram = dram.tile([hq, seq], BF16, tag="dnr")

        tri_sb = cns.tile([P, P], BF16, tag="tri")
        nc.sync.dma_start(tri_sb[:], tri[:])
        iden_sb = cns.tile([P, P], BF16, tag="iden")
        nc.sync.dma_start(iden_sb[:], iden[:])
        ones_sb = cns.tile([P, 1], F32R, tag="ones")
        nc.sync.dma_start(ones_sb[:], ones_col[:])

        kTr = [kp.tile([P, seq], BF16, tag="k", name=f"kTr{g}")
               for g in range(hkv)]
        v_nat = [[vp.tile([P, HEAD_DIM], BF16, tag="v", name=f"v{g}_{t}")
                  for t in range(nkt_total)] for g in range(hkv)]
        # unnormalized attention output tiles, persistent through phase 3
        ao = [[wsb.tile([P, SC], BF16, tag="wsb", name=f"ao{h}_{cc}")
               for cc in range(nch)] for h in range(hq)]

        def rope_drain(dst, psum, cos_c, sinn_c):
            """dst(bf16) = psum*cos + rotate_half(psum)*sin, signed-sin."""
            tmp = ws.tile([P, SC], F32, tag="ws")
            nc.vector.tensor_mul(tmp[0:HH, :], psum[HH:P, :],
                                 sinn_c[0:HH, :])
            nc.vector.tensor_mul(tmp[HH:P, :], psum[0:HH, :],
                                 sinn_c[HH:P, :])
            nc.vector.tensor_mul(dst, psum, cos_c[:])
            nc.vector.tensor_add(dst, dst, tmp[:])

        def emit_loads(c):
            s0 = c * SC
            t = {}
            # chunk 0 is startup-DMA-bound: split weights onto the (idle)
            # gpsimd ring so xT and W stream in parallel.
            wring = nc.gpsimd if c == 0 else nc.sync
            # xT tiles interleaved with the wk quads (K projection is first
            # in the chunk's PE stream), then cos/sin (first needed at the
            # K rope drain), wq hb0 (Q01 is second), wv, then the remaining
            # wq pairs in consumption order.
            t["xT"] = [wsb.tile([P, SC], BF16, tag="wsb", name=f"xT{c}_{i}")
                       for i in range(ndt)]
            t["wk"] = []
            for dt4 in range(ndt // 4):
                for j in range(4):
                    dt = dt4 * 4 + j
                    # chunk 0: alternate xT tiles across the sync and
                    # scalar rings so startup DMA streams at ~2x.
                    xring = nc.scalar if (c == 0 and dt % 2) else nc.sync
                    xring.dma_start(
                        t["xT"][dt][:],
                        xTd[dt * P:(dt + 1) * P, s0:s0 + SC])
                wk = wkvp.tile([P, 4, dkv], BF16, tag="wkv",
                               name=f"wk{c}_{dt4}")
                wring.dma_start(
                    wk[:], wkT[dt4 * 4 * P:(dt4 + 1) * 4 * P, :
                               ].rearrange("(d p) f -> p d f", p=P))
                t["wk"].append(wk)
            t["cos"] = csp.tile([P, SC], F32, tag="cs", name=f"cos{c}")
            nc.sync.dma_start(t["cos"][:], cosT[:, s0:s0 + SC])
            t["sin"] = csp.tile([P, SC], F32, tag="cs", name=f"sin{c}")
            nc.sync.dma_start(t["sin"][:], sinNT[:, s0:s0 + SC])
            t["wq"] = {}
            for hb in range(0, hq, 2):
                for dt4 in range(ndt // 4):
                    wq = wqp.tile([P, 4, 2 * HEAD_DIM], BF16, tag="wq",
                                  name=f"wq{c}_{hb}_{dt4}")
                    wring.dma_start(
                        wq[:], wqT[dt4 * 4 * P:(dt4 + 1) * 4 * P,
                                   hb * HEAD_DIM:(hb + 2) * HEAD_DIM
                                   ].rearrange("(d p) f -> p d f", p=P))
                    t["wq"][(hb, dt4)] = wq
                if hb == 0:
                    t["wv"] = []
                    for dt4 in range(ndt // 4):
                        wv = wkvp.tile([P, 4, dkv], BF16, tag="wkv",
                                       name=f"wv{c}_{dt4}")
                        wring.dma_start(
                            wv[:], wvT[dt4 * 4 * P:(dt4 + 1) * 4 * P, :
                                       ].rearrange("(d p) f -> p d f", p=P))
                        t["wv"].append(wv)
            return t

        def emit_q_pair(c, t, hb, qt):
            """Project q heads hb, hb+1 into per-chunk qt tiles."""
            for i in range(2):
                pq = ps_p.tile([P, SC], F32, tag="p",
                               name=f"pq{c}_{hb}_{i}")
                for dt in range(ndt):
                    wq = t["wq"][(hb, dt // 4)]
                    nc.tensor.matmul(
                        pq[:],
                        wq[:, dt % 4, i * HEAD_DIM:(i + 1) * HEAD_DIM],
                        t["xT"][dt][:],
                        start=(dt == 0), stop=(dt == ndt - 1))
                qt[hb + i] = wsb.tile([P, SC], BF16, tag="wsb",
                                      name=f"qt{c}_{hb + i}")
                rope_drain(qt[hb + i][:], pq[:], t["cos"], t["sin"])

        def emit_proj_k(c, t):
            s0 = c * SC
            for g in range(hkv):
                pk = ps_p.tile([P, SC], F32, tag="p", name=f"pk{c}_{g}")
                for dt in range(ndt):
                    wk = t["wk"][dt // 4]
                    nc.tensor.matmul(
                        pk[:],
                        wk[:, dt % 4, g * HEAD_DIM:(g + 1) * HEAD_DIM],
                        t["xT"][dt][:],
                        start=(dt == 0), stop=(dt == ndt - 1))
                rope_drain(kTr[g][:, s0:s0 + SC], pk[:], t["cos"], t["sin"])

        def emit_proj_v(c, t):
            for g in range(hkv):
                pv = ps_p.tile([P, SC], F32, tag="p", name=f"pv{c}_{g}")
                for dt in range(ndt):
                    wv = t["wv"][dt // 4]
                    nc.tensor.matmul(
                        pv[:],
                        wv[:, dt % 4, g * HEAD_DIM:(g + 1) * HEAD_DIM],
                        t["xT"][dt][:],
                        start=(dt == 0), stop=(dt == ndt - 1))
                vt_sb = wsb.tile([P, SC], BF16, tag="wsb")
                nc.scalar.copy(vt_sb[:], pv[:])
                for st in range(SC // P):
                    pt = ps_a.tile([P, P], BF16, tag="o", bufs=3)
                    nc.tensor.transpose(pt[:], vt_sb[:, st * P:(st + 1) * P],
                                        iden_sb[:])
                    nc.scalar.copy(v_nat[g][c * (SC // P) + st][:], pt[:])

        def emit_normalize_p1(c):
            """1/denom for chunk c + per-head rows to partition 0. DMAs on
            the scalar ring, reciprocal on DVE; no PE involvement."""
            s0 = c * SC
            dn_c = nrm.tile([P, SC], F32, tag="dnf", bufs=2, name=f"dnc{c}")
            nc.scalar.dma_start(dn_c[0:hq, :], dn_dram[:, s0:s0 + SC])
            rc_c = nrm.tile([P, SC], BF16, tag="dnb", bufs=2, name=f"rcc{c}")
            with nc.allow_low_precision(reason="softmax 1/denom in bf16"):
                nc.vector.reciprocal(rc_c[0:hq, :], dn_c[0:hq, :])
            nc.scalar.dma_start(dnr_dram[:, s0:s0 + SC], rc_c[0:hq, :])
            rrows = []
            for h in range(hq):
                rrow = nrm.tile([P, SC], BF16, tag="rr", bufs=6,
                                name=f"rrow{c}_{h}")
                nc.scalar.dma_start(rrow[0:1, :],
                                    dnr_dram[h:h + 1, s0:s0 + SC])
                rrows.append(rrow)
            return rrows

        def emit_normalize_p2(c, rrows):
            """Broadcast each 1/denom row via a PE outer product (ones_row
            [1,128] x rrow [1,512], 216ns) and scale ao in place on DVE."""
            for h in range(hq):
                rbp = ps_a.tile([P, SC], F32, tag="s", bufs=3,
                                name=f"rbp{c}_{h}")
                nc.tensor.matmul(rbp[:], tri_sb[0:1, :], rrows[h][0:1, :],
                                 start=True, stop=True)
                nc.vector.tensor_mul(ao[h][c][:], ao[h][c][:], rbp[:])

        def emit_attention(c, t, qt):
            s0 = c * SC
            nkt = (c + 1) * (SC // P)

            def emit_scores(h, g, kt):
                rr = kt * P - s0
                jlo = max(0, rr)
                pscr = ps_a.tile([P, SC], F32, tag="s", bufs=3,
                                 name=f"pscr{c}_{h}_{kt}")
                nc.tensor.matmul(
                    pscr[:, jlo:SC],
                    kTr[g][:, kt * P:(kt + 1) * P],
                    qt[h][:, jlo:SC],
                    start=True, stop=True)
                return pscr

            def emit_denom(h, acc):
                pd = ps_a.tile([P, SC], F32, tag="s", bufs=3,
                               name=f"pd{c}_{h}")
                nc.tensor.matmul(pd[0:1, :], ones_sb[:], _r(acc[:]),
                                 start=True, stop=True)
                dps = ws.tile([P, SC], F32, tag="ws", name=f"dps{c}_{h}")
                nc.scalar.copy(dps[0:1, :], pd[0:1, :])
                nc.scalar.dma_start(dn_dram[h:h + 1, s0:s0 + SC], dps[0:1, :])

            # remaining Q head-pair projections injected into the stream
            qproj_hooks = {(0, 1): 2, (2, 1): 4, (4, 1): 6}

            # flat (h, kt) stream with scores emitted 2 ahead across
            # head boundaries; denominators deferred into the next head
            items = [(h, kt) for h in range(hq) for kt in range(nkt)]
            pipe = {}

            def sc_ahead(i):
                h2, kt2 = items[i]
                pipe[i] = emit_scores(h2, h2 // nrep, kt2)

            sc_ahead(0)
            if len(items) > 1:
                sc_ahead(1)
            pending = None
            accs = {}
            pos = {}
            for i, (h, kt) in enumerate(items):
                g = h // nrep
                if kt == 0:
                    accs[h] = ws.tile([P, SC], F32, tag="ws",
                                      name=f"acc{c}_{h}")
                    pos[h] = ps_a.tile([P, SC], F32, tag="o", bufs=3,
                                       name=f"po{c}_{h}")
                acc, po = accs[h], pos[h]
                rr = kt * P - s0
                jlo = max(0, rr)
                if i + 2 < len(items):
                    sc_ahead(i + 2)
                hook = qproj_hooks.pop((h, kt), None)
                if hook is not None:
                    emit_q_pair(c, t, hook, qt)
                pscr = pipe.pop(i)
                if kt == 2 and pending is not None:
                    emit_denom(*pending)
                    pending = None
                et = wsb.tile([P, SC], BF16, tag="wsb",
                              name=f"et{c}_{h}_{kt}")
                nc.scalar.activation(
                    et[:, jlo:SC], pscr[:, jlo:SC],
                    mybir.ActivationFunctionType.Exp, scale=scale)
                if rr >= 0:
                    nc.vector.tensor_mul(et[:, jlo:jlo + P],
                                         et[:, jlo:jlo + P], tri_sb[:])
                if kt == 0:
                    nc.vector.tensor_copy(_r(acc[:]), et[:])
                else:
                    nc.vector.tensor_add(_r(acc[:, jlo:SC]),
                                         acc[:, jlo:SC], et[:, jlo:SC])
                nc.tensor.matmul(
                    po[:, jlo:SC],
                    v_nat[g][kt][:],
                    et[:, jlo:SC],
                    start=(kt == 0), stop=(kt == nkt - 1))
                if kt == nkt - 1:
                    if pending is not None:
                        emit_denom(*pending)
                    pending = (h, acc)
                    nc.scalar.copy(ao[h][c][:], po[:])
                    del accs[h], pos[h]
            if pending is not None:
                emit_denom(*pending)

        tiles = emit_loads(0)
        for c in range(nch):
            qt = [None] * hq
            emit_proj_k(c, tiles)
            rrows = emit_normalize_p1(c - 1) if c > 0 else None
            emit_q_pair(c, tiles, 0, qt)
            emit_proj_v(c, tiles)
            if rrows is not None:
                emit_normalize_p2(c - 1, rrows)
            nt = emit_loads(c + 1) if c + 1 < nch else None
            emit_attention(c, tiles, qt)
            tiles = nt

        # --- Phase 3: output projection from SBUF-resident att tiles.
        # Two passes (cc 0..2 then cc 3) so the final chunk's normalization
        # chain overlaps the first pass instead of stalling the PE. Wo tiles
        # prefetch on the (idle) sync DMA ring three iterations ahead.
        passes = ([0, 1], [2, 3]) if nch == 4 else (list(range(nch)),)
        wo_q = []

        def load_wo(m, p_i):
            wo = wop.tile([P, hq, P], BF16, tag="wo", name=f"wo{p_i}_{m}")
            nc.sync.dma_start(
                wo[:], wot[m].rearrange("(o p) f -> p o f", p=P))
            wo_q.append(wo)

        for p_i, cc_pass in enumerate(passes):
            if p_i == len(passes) - 1:
                # last chunk's normalize, hidden under the previous pass
                emit_normalize_p2(nch - 1, emit_normalize_p1(nch - 1))
            for m in range(3 if p_i == 0 else 0):
                load_wo(m, p_i)
            for m in range(dim // P):
                nm = m + 3
                if nm < dim // P:
                    load_wo(nm, p_i)
                elif p_i + 1 < len(passes):
                    load_wo(nm - dim // P, p_i + 1)
                wo = wo_q.pop(0)
                for cc in cc_pass:
                    # alternate free projection banks ("p") with score banks
                    # ("s") for a 6-bank pipeline; drains alternate engines.
                    par = (m + cc) % 2
                    if par == 0:
                        py = ps_p.tile([P, SC], F32, tag="p")
                    else:
                        py = ps_a.tile([P, SC], F32, tag="s", bufs=3)
                    for o in range(hq):
                        nc.tensor.matmul(
                            py[:], wo[:, o, :], ao[o][cc][:],
                            start=(o == 0), stop=(o == hq - 1))
                    yo = ws.tile([P, SC], F32, tag="ws")
                    if par == 0:
                        nc.vector.tensor_copy(yo[:], py[:])
                    else:
                        nc.scalar.copy(yo[:], py[:])
                    nc.sync.dma_start(
                        outT[m * P:(m + 1) * P, cc * SC:(cc + 1) * SC], yo[:])

    nc.compile()
    return nc


def make_core_inputs(data, Wq, Wk, Wv, Wo, cos, sin):
    """Build in_maps for the 8 cores. Core id = 4*b + g."""
    bf = ml_dtypes.bfloat16

    def cbf(a):
        return np.ascontiguousarray(np.asarray(a).astype(bf))

    c = np.ascontiguousarray
    dq = HQ * HEAD_DIM
    dkv = HKV * HEAD_DIM
    dim = Wq.shape[1]
    tri_m = np.triu(np.ones((P, P), dtype=bf))
    iden = np.eye(P, dtype=bf)
    ones_col = np.ones((P, 1), dtype=np.float32)
    cosT = c(cos.T.astype(np.float32))
    sinNT = np.array(sin.T, dtype=np.float32)
    sinNT[0:HH, :] = -sinNT[0:HH, :]
    sinNT = c(sinNT)
    xt_by_batch = [cbf(data[b].T) for b in range(data.shape[0])]
    in_maps = []
    for core in range(N_CORES):
        b, g = divmod(core, 4)
        qs = slice(g * dq, (g + 1) * dq)
        ks = slice(g * dkv, (g + 1) * dkv)
        woT = Wo[:, qs].T                        # [dq, dim]
        wot = cbf(woT.reshape(dq, dim // P, P).transpose(1, 0, 2))
        in_maps.append({
            "xT": xt_by_batch[b],
            "wqT": cbf(Wq[qs, :].T),
            "wkT": cbf(Wk[ks, :].T),
            "wvT": cbf(Wv[ks, :].T),
            "wot": wot,
            "cosT": cosT,
            "sinNT": sinNT,
            "tri": tri_m,
            "iden": iden,
            "ones_col": ones_col,
        })
    return in_maps


_COMPILED = {}


def _get_program():
    key = (SEQ, DIM, HQ, HKV)
    if key not in _COMPILED:
        _COMPILED[key] = build_program()
    return _COMPILED[key]


def run(inputs, trace=False, tmpdir=None, trace_cores=None):
    nc = _get_program()
    in_maps = make_core_inputs(
        inputs["data"], inputs["Wq"], inputs["Wk"], inputs["Wv"],
        inputs["Wo"], inputs["cos"], inputs["sin"])
    kw = {}
    if trace:
        kw = dict(trace=True, tmpdir=tmpdir, trace_cores=trace_cores)
    res = run_bass_kernel_spmd(nc, in_maps, list(range(N_CORES)), **kw)
    B = inputs["data"].shape[0]
    out = np.zeros((B, SEQ, DIM), dtype=np.float32)
    for core in range(N_CORES):
        b = core // 4
        out[b] += res.results[core]["outT"].T
    return out, res


def kernel(data, Wq, Wk, Wv, Wo, cos, sin, mask):
    assert np.asarray(mask).size == 1, "only causal (numel==1) mask supported"
    inputs = {
        "data": np.asarray(data, dtype=np.float32),
        "Wq": np.asarray(Wq, dtype=np.float32),
        "Wk": np.asarray(Wk, dtype=np.float32),
        "Wv": np.asarray(Wv, dtype=np.float32),
        "Wo": np.asarray(Wo, dtype=np.float32),
        "cos": np.asarray(cos, dtype=np.float32),
        "sin": np.asarray(sin, dtype=np.float32),
    }
    out, _ = run(inputs)
    return out
